# revision 6
# baseline (speedup 1.0000x reference)
"""Trainium2 Bass kernel for nn_ConvexGenerator (MoE-routed convex generator).

Expert-parallel with chunk-level load balancing: the 8 classes (with
128-column chunk counts [8,12,16,20,24,28,30,32], total 170) are paired
big-with-small -- (c7,c0) (c6,c1) (c5,c2) (c4,c3) -- and each pair's chunks
are split across 2 cores.  Every core runs the same program shape: slot
group 1 holds up to K1=16 chunks of the pair's big class, group 2 up to
K2=10 chunks of the small class.  Unused slots hold zero Wa/X, which makes
them exact no-ops: exp(ba) is folded multiplicatively into the X columns
host-side (softmax is shift-invariant per class, so ba is first shifted by
its max to keep exp in (0,1]), so a zero X row contributes nothing to
numerator OR denominator (the denominator rides along as a ones-column of
X, also scaled).  Because a class's chunks live on several cores and the
combine is flushed per 8-chunk block, each flush is a *partial* softmax
numerator/denominator; the host sums the partials and divides.  Every
Wa/Xbuf byte is loaded exactly once -- the kernel is HBM-bound at
~360 GB/s/core -- and the per-block combine pipelines behind the logits
instead of tailing after all DMA.

Precision plan (rel-tol 2e-2; measured ~2e-3 end to end):
  cTMU + logits matmuls in fp8e4 with DoubleRow perf mode (2 k-tiles of
  128 per instruction, 0.5 PE cycles/row); exp output and the convex
  combination stay f16 (e/X in fp8 alone costs ~3% output error).
"""

import os

import numpy as np

P = 128
LATENT = 128
C = 8
HID = 1024
D = 512
NMAX = 4096
COUNTS = np.array([1024, 1536, 2048, 2560, 3072, 3584, 3840, 4096])
NCH = COUNTS // P          # chunks per class: [8,12,16,20,24,28,30,32]
PAIRS = [(7, 0), (6, 1), (5, 2), (4, 3)]   # (big, small), chunk-sum 40/42/44/44
K1 = 16                    # slot-1 chunk capacity: max ceil(nch_big / 2)
K2 = 10                    # slot-2 chunk capacity: max ceil(nch_small / 2)
BLK = 8                    # chunks per combine flush block
DXT = 516                  # X (512) | ones | zero pad
OUTW = 514                 # num (512) | den | pad

_cache: dict = {}


def _pad32(n: int) -> int:
    return max(64, -(-n // 32) * 32)


def _blocks(K):
    return [(b, min(BLK, K - b)) for b in range(0, K, BLK)]


def _build(S1: int, S2: int):
    """Per-core Tile program for group sample capacities (S1, S2)."""
    from contextlib import ExitStack

    import concourse.bacc as bacc
    import concourse.mybir as mybir
    import concourse.tile as tile

    f32 = mybir.dt.float32
    f16 = mybir.dt.float16
    f8 = mybir.dt.float8e4
    AF = mybir.ActivationFunctionType
    DR = mybir.MatmulPerfMode.DoubleRow

    nc = bacc.Bacc("TRN2", target_bir_lowering=False, debug=False,
                   enable_asserts=False, num_devices=8)

    groups = [(S1, K1), (S2, K2)]
    SZ = S1 + S2
    # z8 of both groups and the DoubleRow-packed W1 share one [64, 2, *] load
    f8s_d = nc.dram_tensor("f8s", [64, 2, SZ + HID], f8, kind="ExternalInput")
    f32s_d = nc.dram_tensor("f32s", [P, 24], f32, kind="ExternalInput")
    W28_d = nc.dram_tensor("W28", [P, 4, 2, HID], f8, kind="ExternalInput")
    wa_d, x_d, out_d = [], [], []
    for g, (S, K) in enumerate(groups):
        wa_d.append(nc.dram_tensor(f"wa_{g}", [P, K, 4, 2, P], f8, kind="ExternalInput"))
        x_d.append(nc.dram_tensor(f"x_{g}", [P, K, DXT], f16, kind="ExternalInput"))
        out_d.append(nc.dram_tensor(
            f"out_{g}", [len(_blocks(K)), S, OUTW], f16, kind="ExternalOutput"))

    def sblocks(S):
        return [(b, min(512, S - b)) for b in range(0, S, 512)]

    def sgroups(b, bl):
        return [(s, min(256, b + bl - s)) for s in range(b, b + bl, 256)]

    with tile.TileContext(nc) as tc, ExitStack() as ctx:
        consts = ctx.enter_context(tc.tile_pool(name="consts", bufs=1))
        psB = ctx.enter_context(tc.tile_pool(name="psB", bufs=2, space="PSUM"))
        psC = ctx.enter_context(tc.tile_pool(name="psC", bufs=2, space="PSUM"))
        outp = ctx.enter_context(tc.tile_pool(name="outp", bufs=3))

        # ---- input DMAs, in stream order
        f8s_sb = consts.tile([64, 2, SZ + HID], f8)
        nc.sync.dma_start(f8s_sb[:], f8s_d[:])
        f32s_sb = consts.tile([P, 24], f32)
        nc.sync.dma_start(f32s_sb[:], f32s_d[:])
        z8_sb = [f8s_sb[:, :, 0:S1], f8s_sb[:, :, S1:SZ]]
        W18_sb = f8s_sb[:, :, SZ:SZ + HID]
        b1c_sb = [f32s_sb[:, 0:8], f32s_sb[:, 8:16]]
        b2c_sb = f32s_sb[:, 16:24]
        W28_sb = consts.tile([P, 4, 2, HID], f8)
        nc.sync.dma_start(W28_sb[:, 0:2], W28_d[:, 0:2])
        nc.sync.dma_start(W28_sb[:, 2:4], W28_d[:, 2:4])
        wa_sb, x_sb = [], []
        for g, (S, K) in enumerate(groups):
            wa_sb.append(consts.tile([P, K, 4, 2, P], f8, tag=f"wa_{g}", name=f"wa{g}"))
            x_sb.append(consts.tile([P, K, DXT], f16, tag=f"x_{g}", name=f"x{g}"))
        for g, (S, K) in enumerate(groups):
            for k0 in range(0, K, 4):
                k1 = min(k0 + 4, K)
                nc.sync.dma_start(wa_sb[g][:, k0:k1], wa_d[g][:, k0:k1])
                nc.sync.dma_start(x_sb[g][:, k0:k1], x_d[g][:, k0:k1])

        # ---- Phase A: cTMU, two fp8 DoubleRow gelu layers, t in fp8
        h8, t8, e_sb = [], [], []
        for g, (S, K) in enumerate(groups):
            t_h = consts.tile([P, 4, 2, S], f8, tag=f"h8_{g}", name=f"h8{g}")
            h8.append(t_h)
            t_t = consts.tile([P, 4, 2, S], f8, tag=f"t8_{g}", name=f"t8{g}")
            t8.append(t_t)
            t_e = consts.tile([P, K, S], f16, tag=f"e_{g}", name=f"e{g}")
            e_sb.append(t_e)
        for g, (S, K) in enumerate(groups):
            for (b, bl) in sblocks(S):
                for jp in range(4):
                    ph = psB.tile([P, 2, 512], f32, tag="pl")
                    for u in range(2):
                        j = 2 * jp + u
                        for (s0, sl) in sgroups(b, bl):
                            nc.tensor.matmul(ph[:, u, s0 - b:s0 - b + sl],
                                             W18_sb[:, :, j * P:(j + 1) * P],
                                             z8_sb[g][:, :, s0:s0 + sl],
                                             start=True, stop=True, perf_mode=DR,
                                             skip_group_check=True)
                        nc.scalar.activation(h8[g][:, j >> 1, j & 1, b:b + bl],
                                             ph[:, u, :bl], AF.Gelu,
                                             bias=b1c_sb[g][:, j:j + 1])
                for jp in range(4):
                    pt = psB.tile([P, 2, 512], f32, tag="pl")
                    for u in range(2):
                        j = 2 * jp + u
                        for (s0, sl) in sgroups(b, bl):
                            for q in range(4):
                                nc.tensor.matmul(pt[:, u, s0 - b:s0 - b + sl],
                                                 W28_sb[:, q, :, j * P:(j + 1) * P],
                                                 h8[g][:, q, :, s0:s0 + sl],
                                                 start=(q == 0), stop=(q == 3),
                                                 perf_mode=DR,
                                                 skip_group_check=True)
                        nc.scalar.activation(t8[g][:, j >> 1, j & 1, b:b + bl],
                                             pt[:, u, :bl], AF.Gelu,
                                             bias=b2c_sb[:, j:j + 1])

        # ---- Phases B+C interleaved per block: logits (fp8 DR) -> exp
        # (bias-free, chunk-pair batched) -> partial combine, flushed per
        # 8-chunk block so the combine pipelines behind the Wa/X stream.
        for g, (S, K) in enumerate(groups):
            n_st = -(-S // P)
            for (k0, kn) in _blocks(K):
                for ip in range(k0, k0 + kn, 2):
                    npair = min(2, k0 + kn - ip)
                    pl = psB.tile([P, 2, 512], f32, tag="pl")
                    for (b, bl) in sblocks(S):
                        for u in range(npair):
                            for (s0, sl) in sgroups(b, bl):
                                for q in range(4):
                                    nc.tensor.matmul(
                                        pl[:, u, s0 - b:s0 - b + sl],
                                        wa_sb[g][:, ip + u, q, :, :],
                                        t8[g][:, q, :, s0:s0 + sl],
                                        start=(q == 0), stop=(q == 3),
                                        perf_mode=DR, skip_group_check=True)
                        nc.scalar.activation(
                            e_sb[g][:, ip:ip + npair, b:b + bl],
                            pl[:, 0:npair, b:b + bl], AF.Exp)
                for st in range(n_st):
                    sz = min(P, S - st * P)
                    pc = psC.tile([P, 2, 512], f32, tag="pc")
                    for i in range(k0, k0 + kn):
                        lhs = e_sb[g][:, i, st * P:st * P + sz]
                        nc.tensor.matmul(pc[:sz, 0, 0:257], lhs,
                                         x_sb[g][:, i, 0:257],
                                         start=(i == k0), stop=(i == k0 + kn - 1))
                        nc.tensor.matmul(pc[:sz, 1, 0:257], lhs,
                                         x_sb[g][:, i, 257:OUTW],
                                         start=(i == k0), stop=(i == k0 + kn - 1))
                    o = outp.tile([P, OUTW], f16, tag="o")
                    nc.vector.tensor_scalar_mul(o[:sz, 0:257], pc[:sz, 0, 0:257], 1.0)
                    nc.vector.tensor_scalar_mul(o[:sz, 257:OUTW], pc[:sz, 1, 0:257], 1.0)
                    nc.sync.dma_start(
                        out_d[g][k0 // BLK, st * P:st * P + sz, :], o[:sz, :])

    nc.compile()
    return nc


def _get_compiled(key):
    if key not in _cache:
        _cache[key] = _build(*key)
    return _cache[key]


def kernel(z, class_ids, W1, b1, W2, b2, Wa, ba, Xbuf):
    import ml_dtypes
    from concourse.bass_utils import run_bass_kernel_spmd

    f8np = ml_dtypes.float8_e4m3

    def q8(a):
        return np.clip(np.asarray(a, np.float32), -240.0, 240.0).astype(f8np)

    z = np.asarray(z, np.float32)
    class_ids = np.asarray(class_ids).astype(np.int64)
    W1 = np.asarray(W1, np.float32)
    b1 = np.asarray(b1, np.float32)
    W2 = np.asarray(W2, np.float32)
    b2 = np.asarray(b2, np.float32)
    Wa = np.asarray(Wa, np.float32)
    ba = np.asarray(ba, np.float32)
    Xbuf = np.asarray(Xbuf, np.float32)

    B = z.shape[0]
    order = np.argsort(class_ids, kind="stable")
    counts = np.bincount(class_ids, minlength=C)
    idx_by_class = []
    off = 0
    for c in range(C):
        idx_by_class.append(order[off:off + int(counts[c])])
        off += int(counts[c])

    S1 = _pad32(max(int(counts[c]) for c, _ in PAIRS))
    S2 = _pad32(max(int(counts[c]) for _, c in PAIRS))
    nc = _get_compiled((S1, S2))

    # ---- shared weights, packed for DoubleRow k-tiles (2x128 per matmul)
    W18 = q8(W1[:LATENT]).reshape(2, 64, HID).transpose(1, 0, 2)
    W28 = np.ascontiguousarray(
        q8(W2).reshape(4, 2, P, HID).transpose(2, 0, 1, 3))
    # Wa: [C, HID, NMAX] -> [C, p, chunk, q, kt, m]
    Wa8 = np.ascontiguousarray(
        q8(Wa).reshape(C, 4, 2, P, NMAX // P, P).transpose(0, 3, 4, 1, 2, 5))
    # X with exp(ba - max ba) folded into each column (incl. the ones col);
    # softmax is shift-invariant per class so this is exact.
    bshift = np.exp(ba - ba.max(axis=1, keepdims=True))      # [C, NMAX]
    ones = np.ones((C, NMAX, 1), np.float32)
    Xs = np.concatenate(
        [Xbuf[:, :, 0:256], ones, Xbuf[:, :, 256:512], ones],
        axis=2) * bshift[:, :, None]
    X16 = Xs.astype(np.float16).reshape(C, NMAX // P, P, OUTW)

    f32s_by_class = {}
    for c in range(C):
        b1c = np.ascontiguousarray((b1 + W1[LATENT + c]).reshape(8, P).T)
        f32s_by_class[c] = b1c
    b2c = np.ascontiguousarray(b2.reshape(8, P).T)

    def z8_of(c, S):
        n = int(counts[c])
        zp = np.zeros((S, LATENT), f8np)
        zp[:n] = q8(z[idx_by_class[c]])
        return zp.reshape(S, 2, 64).transpose(2, 1, 0)

    def wax_of(c, lo, hi, K):
        wa = np.zeros((P, K, 4, 2, P), f8np)
        wa[:, :hi - lo] = Wa8[c][:, lo:hi]
        x = np.zeros((P, K, DXT), np.float16)
        x[:, :hi - lo, :OUTW] = X16[c][lo:hi].transpose(1, 0, 2)
        return np.ascontiguousarray(wa), np.ascontiguousarray(x)

    in_maps = []
    for p, (cb, cs) in enumerate(PAIRS):
        hb = -(-int(NCH[cb]) // 2)
        hs = -(-int(NCH[cs]) // 2)
        for h in range(2):
            wa1, x1 = wax_of(cb, hb * h, hb if h == 0 else int(NCH[cb]), K1)
            wa2, x2 = wax_of(cs, hs * h, hs if h == 0 else int(NCH[cs]), K2)
            f8s = np.concatenate(
                [z8_of(cb, S1), z8_of(cs, S2), W18], axis=2)
            f32s = np.concatenate(
                [f32s_by_class[cb], f32s_by_class[cs], b2c], axis=1)
            in_maps.append({
                "f8s": np.ascontiguousarray(f8s),
                "f32s": np.ascontiguousarray(f32s),
                "W28": W28,
                "wa_0": wa1, "x_0": x1, "wa_1": wa2, "x_1": x2,
            })

    trace = bool(os.environ.get("BASS_TRACE"))
    res = run_bass_kernel_spmd(
        nc, in_maps, core_ids=list(range(8)),
        trace=trace,
        trace_cores=list(range(8)) if trace else None,
    )
    global _last_results
    _last_results = res

    out = np.zeros((B, D), np.float32)
    for p, (cb, cs) in enumerate(PAIRS):
        for g, c in ((0, cb), (1, cs)):
            n = int(counts[c])
            if n == 0:
                continue
            acc = (res.results[2 * p][f"out_{g}"][:, :n].astype(np.float32).sum(0)
                   + res.results[2 * p + 1][f"out_{g}"][:, :n].astype(np.float32).sum(0))
            num = np.concatenate([acc[:, 0:256], acc[:, 257:513]], axis=1)
            out[idx_by_class[c]] = num / acc[:, 256:257]
    return out


_last_results = None


# revision 7
# speedup vs baseline: 1.0094x; 1.0094x over previous
"""Trainium2 Bass kernel for nn_ConvexGenerator (MoE-routed convex generator).

Expert-parallel with chunk-level load balancing: the 8 classes (with
128-column chunk counts [8,12,16,20,24,28,30,32], total 170) are paired
big-with-small -- (c7,c0) (c6,c1) (c5,c2) (c4,c3) -- and each pair's chunks
are split across 2 cores.  Every core runs the same program shape: slot
group 1 holds up to K1=16 chunks of the pair's big class, group 2 up to
K2=10 chunks of the small class.  Unused slots hold zero Wa/X, which makes
them exact no-ops: exp(ba) is folded multiplicatively into the X columns
host-side (softmax is shift-invariant per class, so ba is first shifted by
its max to keep exp in (0,1]), so a zero X row contributes nothing to
numerator OR denominator (the denominator rides along as a ones-column of
X, also scaled).  Because a class's chunks live on several cores and the
combine is flushed per 8-chunk block, each flush is a *partial* softmax
numerator/denominator; the host sums the partials and divides.  Every
Wa/Xbuf byte is loaded exactly once -- the kernel is HBM-bound at
~360 GB/s/core -- and the per-block combine pipelines behind the logits
instead of tailing after all DMA.

Precision plan (rel-tol 2e-2; measured ~2e-3 end to end):
  cTMU + logits matmuls in fp8e4 with DoubleRow perf mode (2 k-tiles of
  128 per instruction, 0.5 PE cycles/row); exp output and the convex
  combination stay f16 (e/X in fp8 alone costs ~3% output error).
"""

import os

import numpy as np

P = 128
LATENT = 128
C = 8
HID = 1024
D = 512
NMAX = 4096
COUNTS = np.array([1024, 1536, 2048, 2560, 3072, 3584, 3840, 4096])
NCH = COUNTS // P          # chunks per class: [8,12,16,20,24,28,30,32]
PAIRS = [(7, 0), (6, 1), (5, 2), (4, 3)]   # (big, small), chunk-sum 40/42/44/44
K1 = 16                    # slot-1 chunk capacity: max ceil(nch_big / 2)
K2 = 10                    # slot-2 chunk capacity: max ceil(nch_small / 2)
BLK = 8                    # chunks per combine flush block
DXT = 516                  # X (512) | ones | zero pad
OUTW = 514                 # num (512) | den | pad

_cache: dict = {}


def _pad32(n: int) -> int:
    return max(64, -(-n // 32) * 32)


def _blocks(K):
    return [(b, min(BLK, K - b)) for b in range(0, K, BLK)]


def _build(S1: int, S2: int):
    """Per-core Tile program for group sample capacities (S1, S2)."""
    from contextlib import ExitStack

    import concourse.bacc as bacc
    import concourse.mybir as mybir
    import concourse.tile as tile

    f32 = mybir.dt.float32
    f16 = mybir.dt.float16
    f8 = mybir.dt.float8e4
    AF = mybir.ActivationFunctionType
    DR = mybir.MatmulPerfMode.DoubleRow

    nc = bacc.Bacc("TRN2", target_bir_lowering=False, debug=False,
                   enable_asserts=False, num_devices=8)

    groups = [(S1, K1), (S2, K2)]
    SZ = S1 + S2
    # z8 of both groups and the DoubleRow-packed W1 share one [64, 2, *] load
    f8s_d = nc.dram_tensor("f8s", [64, 2, SZ + HID], f8, kind="ExternalInput")
    f32s_d = nc.dram_tensor("f32s", [P, 24], f32, kind="ExternalInput")
    W28_d = nc.dram_tensor("W28", [P, 4, 2, HID], f8, kind="ExternalInput")
    wa_d, x_d, out_d = [], [], []
    for g, (S, K) in enumerate(groups):
        wa_d.append(nc.dram_tensor(f"wa_{g}", [P, K, 4, 2, P], f8, kind="ExternalInput"))
        x_d.append(nc.dram_tensor(f"x_{g}", [P, K, DXT], f16, kind="ExternalInput"))
        out_d.append(nc.dram_tensor(
            f"out_{g}", [len(_blocks(K)), S, OUTW], f16, kind="ExternalOutput"))

    def sblocks(S):
        return [(b, min(512, S - b)) for b in range(0, S, 512)]

    def sgroups(b, bl):
        return [(s, min(256, b + bl - s)) for s in range(b, b + bl, 256)]

    with tile.TileContext(nc) as tc, ExitStack() as ctx:
        consts = ctx.enter_context(tc.tile_pool(name="consts", bufs=1))
        psB = ctx.enter_context(tc.tile_pool(name="psB", bufs=2, space="PSUM"))
        psC = ctx.enter_context(tc.tile_pool(name="psC", bufs=2, space="PSUM"))
        outp = ctx.enter_context(tc.tile_pool(name="outp", bufs=3))

        # ---- input DMAs, in stream order
        f8s_sb = consts.tile([64, 2, SZ + HID], f8)
        nc.sync.dma_start(f8s_sb[:], f8s_d[:])
        f32s_sb = consts.tile([P, 24], f32)
        nc.sync.dma_start(f32s_sb[:], f32s_d[:])
        z8_sb = [f8s_sb[:, :, 0:S1], f8s_sb[:, :, S1:SZ]]
        W18_sb = f8s_sb[:, :, SZ:SZ + HID]
        b1c_sb = [f32s_sb[:, 0:8], f32s_sb[:, 8:16]]
        b2c_sb = f32s_sb[:, 16:24]
        W28_sb = consts.tile([P, 4, 2, HID], f8)
        nc.sync.dma_start(W28_sb[:, 0:2], W28_d[:, 0:2])
        nc.sync.dma_start(W28_sb[:, 2:4], W28_d[:, 2:4])
        wa_sb, x_sb = [], []
        for g, (S, K) in enumerate(groups):
            wa_sb.append(consts.tile([P, K, 4, 2, P], f8, tag=f"wa_{g}", name=f"wa{g}"))
            x_sb.append(consts.tile([P, K, DXT], f16, tag=f"x_{g}", name=f"x{g}"))
        for g, (S, K) in enumerate(groups):
            for k0 in range(0, K, 4):
                k1 = min(k0 + 4, K)
                nc.sync.dma_start(wa_sb[g][:, k0:k1], wa_d[g][:, k0:k1])
                nc.sync.dma_start(x_sb[g][:, k0:k1], x_d[g][:, k0:k1])

        # ---- Phase A: cTMU, two fp8 DoubleRow gelu layers, t in fp8
        h8, t8, e_sb = [], [], []
        for g, (S, K) in enumerate(groups):
            t_h = consts.tile([P, 4, 2, S], f8, tag=f"h8_{g}", name=f"h8{g}")
            h8.append(t_h)
            t_t = consts.tile([P, 4, 2, S], f8, tag=f"t8_{g}", name=f"t8{g}")
            t8.append(t_t)
            t_e = consts.tile([P, K, S], f16, tag=f"e_{g}", name=f"e{g}")
            e_sb.append(t_e)
        for g, (S, K) in enumerate(groups):
            for (b, bl) in sblocks(S):
                for jp in range(4):
                    ph = psB.tile([P, 2, 512], f32, tag="pl")
                    for u in range(2):
                        j = 2 * jp + u
                        for (s0, sl) in sgroups(b, bl):
                            nc.tensor.matmul(ph[:, u, s0 - b:s0 - b + sl],
                                             W18_sb[:, :, j * P:(j + 1) * P],
                                             z8_sb[g][:, :, s0:s0 + sl],
                                             start=True, stop=True, perf_mode=DR,
                                             skip_group_check=True)
                        nc.scalar.activation(h8[g][:, j >> 1, j & 1, b:b + bl],
                                             ph[:, u, :bl], AF.Gelu,
                                             bias=b1c_sb[g][:, j:j + 1])
                for jp in range(4):
                    pt = psB.tile([P, 2, 512], f32, tag="pl")
                    for u in range(2):
                        j = 2 * jp + u
                        for (s0, sl) in sgroups(b, bl):
                            for q in range(4):
                                nc.tensor.matmul(pt[:, u, s0 - b:s0 - b + sl],
                                                 W28_sb[:, q, :, j * P:(j + 1) * P],
                                                 h8[g][:, q, :, s0:s0 + sl],
                                                 start=(q == 0), stop=(q == 3),
                                                 perf_mode=DR,
                                                 skip_group_check=True)
                        nc.scalar.activation(t8[g][:, j >> 1, j & 1, b:b + bl],
                                             pt[:, u, :bl], AF.Gelu,
                                             bias=b2c_sb[:, j:j + 1])

        # ---- Phases B+C, software-pipelined: block b's logit pairs are
        # emitted interleaved with block b-1's combine tiles so the heavy
        # combine matmuls keep PE busy (and at full p-state) while the exps
        # drain on the Activation engine.  Each block flushes a partial
        # num/den that the host sums.
        def emit_pair(g, S, ip, npair):
            pl = psB.tile([P, 2, 512], f32, tag="pl", name="pl")
            for (b, bl) in sblocks(S):
                for u in range(npair):
                    for (s0, sl) in sgroups(b, bl):
                        for q in range(4):
                            nc.tensor.matmul(
                                pl[:, u, s0 - b:s0 - b + sl],
                                wa_sb[g][:, ip + u, q, :, :],
                                t8[g][:, q, :, s0:s0 + sl],
                                start=(q == 0), stop=(q == 3),
                                perf_mode=DR, skip_group_check=True)
                nc.scalar.activation(
                    e_sb[g][:, ip:ip + npair, b:b + bl],
                    pl[:, 0:npair, b:b + bl], AF.Exp)

        def emit_ctile(g, S, k0, kn, st):
            sz = min(P, S - st * P)
            pc = psC.tile([P, 2, 512], f32, tag="pc", name="pc")
            for i in range(k0, k0 + kn):
                lhs = e_sb[g][:, i, st * P:st * P + sz]
                nc.tensor.matmul(pc[:sz, 0, 0:257], lhs,
                                 x_sb[g][:, i, 0:257],
                                 start=(i == k0), stop=(i == k0 + kn - 1))
                nc.tensor.matmul(pc[:sz, 1, 0:257], lhs,
                                 x_sb[g][:, i, 257:OUTW],
                                 start=(i == k0), stop=(i == k0 + kn - 1))
            o = outp.tile([P, OUTW], f16, tag="o", name="o")
            nc.vector.tensor_scalar_mul(o[:sz, 0:257], pc[:sz, 0, 0:257], 1.0)
            nc.vector.tensor_scalar_mul(o[:sz, 257:OUTW], pc[:sz, 1, 0:257], 1.0)
            nc.sync.dma_start(
                out_d[g][k0 // BLK, st * P:st * P + sz, :], o[:sz, :])

        pending = None   # (g, S, k0, kn) combine work carried across blocks
        for g, (S, K) in enumerate(groups):
            for (k0, kn) in _blocks(K):
                pairs = [(ip, min(2, k0 + kn - ip))
                         for ip in range(k0, k0 + kn, 2)]
                ct = ([(pending, st) for st in range(-(-pending[1] // P))]
                      if pending else [])
                pi = ci = 0
                while pi < len(pairs) or ci < len(ct):
                    if pi < len(pairs):
                        emit_pair(g, S, *pairs[pi])
                        pi += 1
                    if ci < len(ct):
                        (pg, pS, pk0, pkn), st = ct[ci]
                        emit_ctile(pg, pS, pk0, pkn, st)
                        ci += 1
                pending = (g, S, k0, kn)
        pg, pS, pk0, pkn = pending
        for st in range(-(-pS // P)):
            emit_ctile(pg, pS, pk0, pkn, st)

    nc.compile()
    return nc


def _get_compiled(key):
    if key not in _cache:
        _cache[key] = _build(*key)
    return _cache[key]


def kernel(z, class_ids, W1, b1, W2, b2, Wa, ba, Xbuf):
    import ml_dtypes
    from concourse.bass_utils import run_bass_kernel_spmd

    f8np = ml_dtypes.float8_e4m3

    def q8(a):
        return np.clip(np.asarray(a, np.float32), -240.0, 240.0).astype(f8np)

    z = np.asarray(z, np.float32)
    class_ids = np.asarray(class_ids).astype(np.int64)
    W1 = np.asarray(W1, np.float32)
    b1 = np.asarray(b1, np.float32)
    W2 = np.asarray(W2, np.float32)
    b2 = np.asarray(b2, np.float32)
    Wa = np.asarray(Wa, np.float32)
    ba = np.asarray(ba, np.float32)
    Xbuf = np.asarray(Xbuf, np.float32)

    B = z.shape[0]
    order = np.argsort(class_ids, kind="stable")
    counts = np.bincount(class_ids, minlength=C)
    idx_by_class = []
    off = 0
    for c in range(C):
        idx_by_class.append(order[off:off + int(counts[c])])
        off += int(counts[c])

    S1 = _pad32(max(int(counts[c]) for c, _ in PAIRS))
    S2 = _pad32(max(int(counts[c]) for _, c in PAIRS))
    nc = _get_compiled((S1, S2))

    # ---- shared weights, packed for DoubleRow k-tiles (2x128 per matmul)
    W18 = q8(W1[:LATENT]).reshape(2, 64, HID).transpose(1, 0, 2)
    W28 = np.ascontiguousarray(
        q8(W2).reshape(4, 2, P, HID).transpose(2, 0, 1, 3))
    # Wa: [C, HID, NMAX] -> [C, p, chunk, q, kt, m]
    Wa8 = np.ascontiguousarray(
        q8(Wa).reshape(C, 4, 2, P, NMAX // P, P).transpose(0, 3, 4, 1, 2, 5))
    # X with exp(ba - max ba) folded into each column (incl. the ones col);
    # softmax is shift-invariant per class so this is exact.
    bshift = np.exp(ba - ba.max(axis=1, keepdims=True))      # [C, NMAX]
    ones = np.ones((C, NMAX, 1), np.float32)
    Xs = np.concatenate(
        [Xbuf[:, :, 0:256], ones, Xbuf[:, :, 256:512], ones],
        axis=2) * bshift[:, :, None]
    X16 = Xs.astype(np.float16).reshape(C, NMAX // P, P, OUTW)

    f32s_by_class = {}
    for c in range(C):
        b1c = np.ascontiguousarray((b1 + W1[LATENT + c]).reshape(8, P).T)
        f32s_by_class[c] = b1c
    b2c = np.ascontiguousarray(b2.reshape(8, P).T)

    def z8_of(c, S):
        n = int(counts[c])
        zp = np.zeros((S, LATENT), f8np)
        zp[:n] = q8(z[idx_by_class[c]])
        return zp.reshape(S, 2, 64).transpose(2, 1, 0)

    def wax_of(c, lo, hi, K):
        wa = np.zeros((P, K, 4, 2, P), f8np)
        wa[:, :hi - lo] = Wa8[c][:, lo:hi]
        x = np.zeros((P, K, DXT), np.float16)
        x[:, :hi - lo, :OUTW] = X16[c][lo:hi].transpose(1, 0, 2)
        return np.ascontiguousarray(wa), np.ascontiguousarray(x)

    in_maps = []
    for p, (cb, cs) in enumerate(PAIRS):
        hb = -(-int(NCH[cb]) // 2)
        hs = -(-int(NCH[cs]) // 2)
        for h in range(2):
            wa1, x1 = wax_of(cb, hb * h, hb if h == 0 else int(NCH[cb]), K1)
            wa2, x2 = wax_of(cs, hs * h, hs if h == 0 else int(NCH[cs]), K2)
            f8s = np.concatenate(
                [z8_of(cb, S1), z8_of(cs, S2), W18], axis=2)
            f32s = np.concatenate(
                [f32s_by_class[cb], f32s_by_class[cs], b2c], axis=1)
            in_maps.append({
                "f8s": np.ascontiguousarray(f8s),
                "f32s": np.ascontiguousarray(f32s),
                "W28": W28,
                "wa_0": wa1, "x_0": x1, "wa_1": wa2, "x_1": x2,
            })

    trace = bool(os.environ.get("BASS_TRACE"))
    res = run_bass_kernel_spmd(
        nc, in_maps, core_ids=list(range(8)),
        trace=trace,
        trace_cores=list(range(8)) if trace else None,
    )
    global _last_results
    _last_results = res

    out = np.zeros((B, D), np.float32)
    for p, (cb, cs) in enumerate(PAIRS):
        for g, c in ((0, cb), (1, cs)):
            n = int(counts[c])
            if n == 0:
                continue
            acc = (res.results[2 * p][f"out_{g}"][:, :n].astype(np.float32).sum(0)
                   + res.results[2 * p + 1][f"out_{g}"][:, :n].astype(np.float32).sum(0))
            num = np.concatenate([acc[:, 0:256], acc[:, 257:513]], axis=1)
            out[idx_by_class[c]] = num / acc[:, 256:257]
    return out


_last_results = None


# revision 8
# speedup vs baseline: 1.0817x; 1.0716x over previous
"""Trainium2 Bass kernel for nn_ConvexGenerator (MoE-routed convex generator).

Expert-parallel with chunk-level load balancing: the 8 classes (with
128-column chunk counts [8,12,16,20,24,28,30,32], total 170) are paired
big-with-small -- (c7,c0) (c6,c1) (c5,c2) (c4,c3) -- and each pair's chunks
are split across 2 cores.  Every core runs the same program shape: slot
group 1 holds up to K1=16 chunks of the pair's big class, group 2 up to
K2=10 chunks of the small class.  Unused slots hold zero Wa/X, which makes
them exact no-ops: exp(ba) is folded multiplicatively into the X columns
host-side (softmax is shift-invariant per class, so ba is first shifted by
its max to keep exp in (0,1]), so a zero X row contributes nothing to
numerator OR denominator (the denominator rides along as a ones-column of
X, also scaled).  Because a class's chunks live on several cores and the
combine is flushed per 8-chunk block, each flush is a *partial* softmax
numerator/denominator; the host sums the partials and divides.  Every
Wa/Xbuf byte is loaded exactly once -- the kernel is HBM-bound at
~360 GB/s/core -- and the per-block combine pipelines behind the logits
instead of tailing after all DMA.

Precision plan (rel-tol 2e-2; measured ~2e-3 end to end):
  cTMU + logits matmuls in fp8e4 with DoubleRow perf mode (2 k-tiles of
  128 per instruction, 0.5 PE cycles/row); exp output and the convex
  combination stay f16 (e/X in fp8 alone costs ~3% output error).
"""

import os

import numpy as np

P = 128
LATENT = 128
C = 8
HID = 1024
D = 512
NMAX = 4096
COUNTS = np.array([1024, 1536, 2048, 2560, 3072, 3584, 3840, 4096])
NCH = COUNTS // P          # chunks per class: [8,12,16,20,24,28,30,32]
PAIRS = [(7, 0), (6, 1), (5, 2), (4, 3)]   # (big, small), chunk-sum 40/42/44/44
K1 = 16                    # slot-1 chunk capacity: max ceil(nch_big / 2)
K2 = 10                    # slot-2 chunk capacity: max ceil(nch_small / 2)
BLK = 8                    # chunks per combine flush block
DXT = 516                  # X (512) | ones | zero pad
OUTW = 514                 # num (512) | den | pad

_cache: dict = {}


def _pad32(n: int) -> int:
    return max(64, -(-n // 32) * 32)


def _blocks(K):
    return [(b, min(BLK, K - b)) for b in range(0, K, BLK)]


def _build(S1: int, S2: int):
    """Per-core Tile program for group sample capacities (S1, S2)."""
    from contextlib import ExitStack

    import concourse.bacc as bacc
    import concourse.mybir as mybir
    import concourse.tile as tile

    f32 = mybir.dt.float32
    f16 = mybir.dt.float16
    f8 = mybir.dt.float8e4
    AF = mybir.ActivationFunctionType
    DR = mybir.MatmulPerfMode.DoubleRow

    nc = bacc.Bacc("TRN2", target_bir_lowering=False, debug=False,
                   enable_asserts=False, num_devices=8)

    groups = [(S1, K1), (S2, K2)]
    SZ = S1 + S2
    # z8 of both groups and the DoubleRow-packed W1 share one [64, 2, *] load
    f8s_d = nc.dram_tensor("f8s", [64, 2, SZ + HID], f8, kind="ExternalInput")
    f32s_d = nc.dram_tensor("f32s", [P, 24], f32, kind="ExternalInput")
    W28_d = nc.dram_tensor("W28", [P, 4, 2, HID], f8, kind="ExternalInput")
    wa_d, x_d, out_d = [], [], []
    for g, (S, K) in enumerate(groups):
        wa_d.append(nc.dram_tensor(f"wa_{g}", [P, K, 4, 2, P], f8, kind="ExternalInput"))
        x_d.append(nc.dram_tensor(f"x_{g}", [P, K, DXT], f16, kind="ExternalInput"))
        out_d.append(nc.dram_tensor(
            f"out_{g}", [len(_blocks(K)), S, OUTW], f16, kind="ExternalOutput"))

    def sblocks(S):
        return [(b, min(512, S - b)) for b in range(0, S, 512)]

    def sgroups(b, bl):
        return [(s, min(256, b + bl - s)) for s in range(b, b + bl, 256)]

    with tile.TileContext(nc) as tc, ExitStack() as ctx:
        consts = ctx.enter_context(tc.tile_pool(name="consts", bufs=1))
        psB = ctx.enter_context(tc.tile_pool(name="psB", bufs=4, space="PSUM"))
        psC = ctx.enter_context(tc.tile_pool(name="psC", bufs=2, space="PSUM"))
        outp = ctx.enter_context(tc.tile_pool(name="outp", bufs=3))

        # ---- input DMAs, in stream order
        f8s_sb = consts.tile([64, 2, SZ + HID], f8)
        nc.sync.dma_start(f8s_sb[:], f8s_d[:])
        f32s_sb = consts.tile([P, 24], f32)
        nc.sync.dma_start(f32s_sb[:], f32s_d[:])
        z8_sb = [f8s_sb[:, :, 0:S1], f8s_sb[:, :, S1:SZ]]
        W18_sb = f8s_sb[:, :, SZ:SZ + HID]
        b1c_sb = [f32s_sb[:, 0:8], f32s_sb[:, 8:16]]
        b2c_sb = f32s_sb[:, 16:24]
        W28_sb = consts.tile([P, 4, 2, HID], f8)
        nc.sync.dma_start(W28_sb[:, 0:2], W28_d[:, 0:2])
        nc.sync.dma_start(W28_sb[:, 2:4], W28_d[:, 2:4])
        wa_sb, x_sb = [], []
        for g, (S, K) in enumerate(groups):
            wa_sb.append(consts.tile([P, K, 4, 2, P], f8, tag=f"wa_{g}", name=f"wa{g}"))
            x_sb.append(consts.tile([P, K, DXT], f16, tag=f"x_{g}", name=f"x{g}"))
        for g, (S, K) in enumerate(groups):
            for k0 in range(0, K, 4):
                k1 = min(k0 + 4, K)
                nc.sync.dma_start(wa_sb[g][:, k0:k1], wa_d[g][:, k0:k1])
                nc.sync.dma_start(x_sb[g][:, k0:k1], x_d[g][:, k0:k1])

        # ---- Phase A: cTMU, two fp8 DoubleRow gelu layers, t in fp8
        h8, t8, e_sb = [], [], []
        for g, (S, K) in enumerate(groups):
            t_h = consts.tile([P, 4, 2, S], f8, tag=f"h8_{g}", name=f"h8{g}")
            h8.append(t_h)
            t_t = consts.tile([P, 4, 2, S], f8, tag=f"t8_{g}", name=f"t8{g}")
            t8.append(t_t)
            t_e = consts.tile([P, K, S], f16, tag=f"e_{g}", name=f"e{g}")
            e_sb.append(t_e)
        for g, (S, K) in enumerate(groups):
            for (b, bl) in sblocks(S):
                for j in range(8):
                    ph = psB.tile([P, 512], f32, tag="pl", name="ph")
                    for (s0, sl) in sgroups(b, bl):
                        nc.tensor.matmul(ph[:, s0 - b:s0 - b + sl],
                                         W18_sb[:, :, j * P:(j + 1) * P],
                                         z8_sb[g][:, :, s0:s0 + sl],
                                         start=True, stop=True, perf_mode=DR,
                                         skip_group_check=True)
                    nc.scalar.activation(h8[g][:, j >> 1, j & 1, b:b + bl],
                                         ph[:, :bl], AF.Gelu,
                                         bias=b1c_sb[g][:, j:j + 1])
                for j in range(8):
                    pt = psB.tile([P, 512], f32, tag="pl", name="pt")
                    for (s0, sl) in sgroups(b, bl):
                        for q in range(4):
                            nc.tensor.matmul(pt[:, s0 - b:s0 - b + sl],
                                             W28_sb[:, q, :, j * P:(j + 1) * P],
                                             h8[g][:, q, :, s0:s0 + sl],
                                             start=(q == 0), stop=(q == 3),
                                             perf_mode=DR,
                                             skip_group_check=True)
                    nc.scalar.activation(t8[g][:, j >> 1, j & 1, b:b + bl],
                                         pt[:, :bl], AF.Gelu,
                                         bias=b2c_sb[:, j:j + 1])

        # ---- Phases B+C, software-pipelined at chunk granularity: block
        # b's per-chunk logit+exp ops interleave with block b-1's combine
        # units (2 matmuls each, sample-tile-major) so PE stays busy at
        # full p-state while exps drain on the Activation engine, and the
        # exp ladder (psB bufs=4 deep) keeps advancing.  Each block flushes
        # a partial num/den that the host sums.
        def emit_bchunk(g, S, i):
            pl = psB.tile([P, 512], f32, tag="pl", name="pl")
            for (b, bl) in sblocks(S):
                for (s0, sl) in sgroups(b, bl):
                    for q in range(4):
                        nc.tensor.matmul(pl[:, s0 - b:s0 - b + sl],
                                         wa_sb[g][:, i, q, :, :],
                                         t8[g][:, q, :, s0:s0 + sl],
                                         start=(q == 0), stop=(q == 3),
                                         perf_mode=DR, skip_group_check=True)
                nc.scalar.activation(e_sb[g][:, i, b:b + bl], pl[:, :bl],
                                     AF.Exp)

        pc_cur = [None]

        def emit_cunit(g, S, k0, kn, st, i):
            sz = min(P, S - st * P)
            if i == k0:
                pc_cur[0] = psC.tile([P, 2, 512], f32, tag="pc", name="pc")
            pc = pc_cur[0]
            lhs = e_sb[g][:, i, st * P:st * P + sz]
            nc.tensor.matmul(pc[:sz, 0, 0:257], lhs, x_sb[g][:, i, 0:257],
                             start=(i == k0), stop=(i == k0 + kn - 1))
            nc.tensor.matmul(pc[:sz, 1, 0:257], lhs, x_sb[g][:, i, 257:OUTW],
                             start=(i == k0), stop=(i == k0 + kn - 1))
            if i == k0 + kn - 1:
                o = outp.tile([P, OUTW], f16, tag="o", name="o")
                nc.vector.tensor_scalar_mul(o[:sz, 0:257],
                                            pc[:sz, 0, 0:257], 1.0)
                nc.vector.tensor_scalar_mul(o[:sz, 257:OUTW],
                                            pc[:sz, 1, 0:257], 1.0)
                nc.sync.dma_start(
                    out_d[g][k0 // BLK, st * P:st * P + sz, :], o[:sz, :])

        def c_ops_of(g, S, k0, kn):
            ops = []
            for st in range(-(-S // P)):
                for i in range(k0, k0 + kn):
                    ops.append((g, S, k0, kn, st, i))
            return ops

        pending = None   # (g, S, k0, kn): combine work carried across blocks
        for g, (S, K) in enumerate(groups):
            for (k0, kn) in _blocks(K):
                c_ops = c_ops_of(*pending) if pending else []
                ci = 0
                for u, i in enumerate(range(k0, k0 + kn)):
                    emit_bchunk(g, S, i)
                    ct = (u + 1) * len(c_ops) // kn
                    while ci < ct:
                        emit_cunit(*c_ops[ci])
                        ci += 1
                while ci < len(c_ops):
                    emit_cunit(*c_ops[ci])
                    ci += 1
                pending = (g, S, k0, kn)
        for op in c_ops_of(*pending):
            emit_cunit(*op)

    nc.compile()
    return nc


def _get_compiled(key):
    if key not in _cache:
        _cache[key] = _build(*key)
    return _cache[key]


def kernel(z, class_ids, W1, b1, W2, b2, Wa, ba, Xbuf):
    import ml_dtypes
    from concourse.bass_utils import run_bass_kernel_spmd

    f8np = ml_dtypes.float8_e4m3

    def q8(a):
        return np.clip(np.asarray(a, np.float32), -240.0, 240.0).astype(f8np)

    z = np.asarray(z, np.float32)
    class_ids = np.asarray(class_ids).astype(np.int64)
    W1 = np.asarray(W1, np.float32)
    b1 = np.asarray(b1, np.float32)
    W2 = np.asarray(W2, np.float32)
    b2 = np.asarray(b2, np.float32)
    Wa = np.asarray(Wa, np.float32)
    ba = np.asarray(ba, np.float32)
    Xbuf = np.asarray(Xbuf, np.float32)

    B = z.shape[0]
    order = np.argsort(class_ids, kind="stable")
    counts = np.bincount(class_ids, minlength=C)
    idx_by_class = []
    off = 0
    for c in range(C):
        idx_by_class.append(order[off:off + int(counts[c])])
        off += int(counts[c])

    S1 = _pad32(max(int(counts[c]) for c, _ in PAIRS))
    S2 = _pad32(max(int(counts[c]) for _, c in PAIRS))
    nc = _get_compiled((S1, S2))

    # ---- shared weights, packed for DoubleRow k-tiles (2x128 per matmul)
    W18 = q8(W1[:LATENT]).reshape(2, 64, HID).transpose(1, 0, 2)
    W28 = np.ascontiguousarray(
        q8(W2).reshape(4, 2, P, HID).transpose(2, 0, 1, 3))
    # Wa: [C, HID, NMAX] -> [C, p, chunk, q, kt, m]
    Wa8 = np.ascontiguousarray(
        q8(Wa).reshape(C, 4, 2, P, NMAX // P, P).transpose(0, 3, 4, 1, 2, 5))
    # X with exp(ba - max ba) folded into each column (incl. the ones col);
    # softmax is shift-invariant per class so this is exact.
    bshift = np.exp(ba - ba.max(axis=1, keepdims=True))      # [C, NMAX]
    ones = np.ones((C, NMAX, 1), np.float32)
    Xs = np.concatenate(
        [Xbuf[:, :, 0:256], ones, Xbuf[:, :, 256:512], ones],
        axis=2) * bshift[:, :, None]
    X16 = Xs.astype(np.float16).reshape(C, NMAX // P, P, OUTW)

    f32s_by_class = {}
    for c in range(C):
        b1c = np.ascontiguousarray((b1 + W1[LATENT + c]).reshape(8, P).T)
        f32s_by_class[c] = b1c
    b2c = np.ascontiguousarray(b2.reshape(8, P).T)

    def z8_of(c, S):
        n = int(counts[c])
        zp = np.zeros((S, LATENT), f8np)
        zp[:n] = q8(z[idx_by_class[c]])
        return zp.reshape(S, 2, 64).transpose(2, 1, 0)

    def wax_of(c, lo, hi, K):
        wa = np.zeros((P, K, 4, 2, P), f8np)
        wa[:, :hi - lo] = Wa8[c][:, lo:hi]
        x = np.zeros((P, K, DXT), np.float16)
        x[:, :hi - lo, :OUTW] = X16[c][lo:hi].transpose(1, 0, 2)
        return np.ascontiguousarray(wa), np.ascontiguousarray(x)

    in_maps = []
    for p, (cb, cs) in enumerate(PAIRS):
        hb = -(-int(NCH[cb]) // 2)
        hs = -(-int(NCH[cs]) // 2)
        for h in range(2):
            wa1, x1 = wax_of(cb, hb * h, hb if h == 0 else int(NCH[cb]), K1)
            wa2, x2 = wax_of(cs, hs * h, hs if h == 0 else int(NCH[cs]), K2)
            f8s = np.concatenate(
                [z8_of(cb, S1), z8_of(cs, S2), W18], axis=2)
            f32s = np.concatenate(
                [f32s_by_class[cb], f32s_by_class[cs], b2c], axis=1)
            in_maps.append({
                "f8s": np.ascontiguousarray(f8s),
                "f32s": np.ascontiguousarray(f32s),
                "W28": W28,
                "wa_0": wa1, "x_0": x1, "wa_1": wa2, "x_1": x2,
            })

    trace = bool(os.environ.get("BASS_TRACE"))
    res = run_bass_kernel_spmd(
        nc, in_maps, core_ids=list(range(8)),
        trace=trace,
        trace_cores=list(range(8)) if trace else None,
    )
    global _last_results
    _last_results = res

    out = np.zeros((B, D), np.float32)
    for p, (cb, cs) in enumerate(PAIRS):
        for g, c in ((0, cb), (1, cs)):
            n = int(counts[c])
            if n == 0:
                continue
            acc = (res.results[2 * p][f"out_{g}"][:, :n].astype(np.float32).sum(0)
                   + res.results[2 * p + 1][f"out_{g}"][:, :n].astype(np.float32).sum(0))
            num = np.concatenate([acc[:, 0:256], acc[:, 257:513]], axis=1)
            out[idx_by_class[c]] = num / acc[:, 256:257]
    return out


_last_results = None
